# revision 20
# baseline (speedup 1.0000x reference)
"""Trainium2 Bass kernel for 3D neighborhood attention (sparse_attention).

Problem: q,k [1,40,40,40,48] fp32, rpb [8,3,3,3]; out [1,24,40,40,40].
Per voxel x: logits[h,kk] = scale * <q[x,h,:], k[x+off_kk,h,:]> + rpb[h,kk]
(zero-padded k at boundaries, kk over 3x3x3 offsets), p = softmax over kk,
out[x,h,:] = sum_kk p[h,kk] * off_kk  (constant integer offsets as values).

Sharding: spatial-parallel over H (40 -> 8 slabs of 5). Each core gets its
q slab plus a host-side im2col of the 27 shifted k views for its slab
(halo handled on host): on-core everything is token-parallel with tokens
on SBUF partitions (9 tokens per partition, 7 tiles of 1152 tokens).

Engine split:
 - DVE: QK products in fp16 (2x_1P packed mode), then the factorized
   (di -> dj -> dl) fold tree over exp'd logits that yields the softmax
   denominator and the three directional numerators (values are the
   constant offsets in {-1,0,1}^3), plus reciprocal + final scale.
 - TensorE: the d-reduction of the products as accumulating identity
   matmuls into PSUM (6 d-blocks + an rpb seed per 432-wide chunk), so
   logits materialize in PSUM in fp32 for free.
 - ScalarE: exp straight from PSUM into fp16 SBUF.
 The DVE fold tree for tile i is emitted during tile i+1 (software
 pipelining) so the in-order DVE stream never stalls on the
 TensorE->ScalarE chain; the output DMA is dispatched from the GpSimd
 DGE queue so it never head-of-line-blocks the input DMAs on the sync
 queue; tile 0's k-neighborhood DMA is split per d-block to shorten the
 initial ramp.
"""

import numpy as np

import concourse.bass as bass
import concourse.tile as tile
from concourse import bacc, mybir
from concourse.bass_utils import run_bass_kernel_spmd

F16 = np.float16

NH = 8
HD = 6
DIM = NH * HD
KS = 3
NT = KS**3  # 27
SCALE = HD**-0.5
H = W = T = 40
N_CORES = 8
SLAB = H // N_CORES          # 5 rows of H per core
TOK = SLAB * W * T           # 8000 tokens per core
P = 128
TPP = 9                      # tokens per partition per tile
TILES = 7                    # 7 * 128 * 9 = 8064 >= 8000
TOKP = TILES * P * TPP       # 8064
FKH = NT * NH                # 216 logits per token
NCHUNK = 4                   # PSUM chunks per tile
CHUNK = TPP * FKH // NCHUNK  # 486 logits (<=512 fp32, one PSUM bank)

_prog_cache = {}


def _build_program():
    fp16 = mybir.dt.float16
    fp32 = mybir.dt.float32
    nc = bacc.Bacc("TRN2", target_bir_lowering=False, debug=False,
                   num_devices=N_CORES)
    # free layouts (per partition):
    #   qs : (d6, j8, h8)             = 384
    #   kn : (d6, j8, kk27, h8)       = 10368
    #   rpb: (kk27, h8)               = 216
    #   out: (o3, j8, h8)             = 192
    qs = nc.dram_tensor("qs", [TILES, P, HD * TPP * NH], fp16,
                        kind="ExternalInput").ap()
    kn = nc.dram_tensor("kn", [TILES, P, HD * TPP * NT * NH], fp16,
                        kind="ExternalInput").ap()
    rpbt = nc.dram_tensor("rpbt", [P, FKH], fp16, kind="ExternalInput").ap()
    ident_in = nc.dram_tensor("ident", [P, P], fp16,
                              kind="ExternalInput").ap()
    out = nc.dram_tensor("out", [TILES, P, 3 * TPP * NH], fp32,
                         kind="ExternalOutput").ap()

    J = TPP
    NJH = TPP * NT * NH         # 1728 logits per partition
    D3 = 3 * NJH                # 5184: half of the products

    with tile.TileContext(nc) as tc:
        with (
            tc.tile_pool(name="consts", bufs=1) as cpool,
            tc.tile_pool(name="kin", bufs=4) as kpool,
            tc.tile_pool(name="qin", bufs=2) as qpool,
            tc.tile_pool(name="prod", bufs=2) as ppool,
            tc.tile_pool(name="psum", bufs=8, space="PSUM") as pspool,
            tc.tile_pool(name="expv", bufs=2) as epool,
            tc.tile_pool(name="l1", bufs=2) as l1pool,
            tc.tile_pool(name="l3", bufs=2) as l3pool,
            tc.tile_pool(name="tt", bufs=2) as ttpool,
            tc.tile_pool(name="small", bufs=8) as spool,
            tc.tile_pool(name="outp", bufs=2) as opool,
        ):
            rpb_sb = cpool.tile([P, FKH], fp16)
            nc.sync.dma_start(rpb_sb[:], rpbt[:])
            ident = cpool.tile([P, P], fp16)
            nc.sync.dma_start(ident[:], ident_in[:])
            # rpb replicated across all tokens of a tile; chunk seeds are
            # 486-wide slices of this
            rpb_rep = cpool.tile([P, TPP * FKH], fp16)
            nc.vector.tensor_copy(
                rpb_rep[:].rearrange("p (j f) -> p j f", j=TPP),
                rpb_sb[:].unsqueeze(1).broadcast_to([P, TPP, FKH]))

            state = {}

            def emit_front(ti):
                """DMA + QK mul + TensorE d-fold + exp for tile ti."""
                qt = qpool.tile([P, HD * TPP * NH], fp16)
                nc.sync.dma_start(qt[:], qs[ti])
                # tile 0 loads per-d-block so the first mul starts as soon
                # as possible; later tiles use one fused load+mul (fewer
                # per-op bubbles)
                nparts = HD if ti == 0 else 1
                dpp = HD // nparts            # d-blocks per part
                PSZ = dpp * NJH
                ktiles = []
                for pi in range(nparts):
                    ktp = kpool.tile([P, PSZ], fp16)
                    nc.sync.dma_start(
                        ktp[:], kn[ti, :, pi * PSZ:(pi + 1) * PSZ])
                    ktiles.append(ktp)

                pt = ppool.tile([P, HD * NJH], fp16)
                qv = qt[:].rearrange("p (d j h) -> p d j h", d=HD, j=J)
                for pi, ktile in enumerate(ktiles):
                    q_b = (qv[:, dpp * pi:dpp * (pi + 1)]
                           .unsqueeze(3).broadcast_to([P, dpp, J, NT, NH]))
                    nc.vector.tensor_mul(
                        pt[:, pi * PSZ:(pi + 1) * PSZ].rearrange(
                            "p (d j kk h) -> p d j kk h", d=dpp, j=J, kk=NT),
                        ktile[:].rearrange(
                            "p (d j kk h) -> p d j kk h", d=dpp, j=J, kk=NT),
                        q_b,
                    )
                # logits into PSUM: per chunk, seed with rpb then accumulate
                # the 6 d-blocks via identity matmuls
                et = epool.tile([P, NJH], fp16)
                for c in range(NCHUNK):
                    pc = pspool.tile([P, CHUNK], fp32)
                    nc.tensor.matmul(
                        pc[:], ident[:],
                        rpb_rep[:, c * CHUNK:(c + 1) * CHUNK],
                        start=True, stop=False)
                    for b in range(HD):
                        nc.tensor.matmul(
                            pc[:], ident[:],
                            pt[:, b * NJH + c * CHUNK:
                                b * NJH + (c + 1) * CHUNK],
                            start=False, stop=(b == HD - 1))
                    nc.scalar.activation(et[:, c * CHUNK:(c + 1) * CHUNK],
                                         pc[:],
                                         mybir.ActivationFunctionType.Exp)
                state[ti] = et

            def emit_back(ti):
                """DVE fold tree + out for tile ti (runs one tile late)."""
                et = state.pop(ti)
                # level 1 (contract di): a0 = sum_di E, a1 = E[di2]-E[di0]
                ev = et[:].rearrange("p (j di r) -> p j di r", j=J, di=KS)
                tt = ttpool.tile([P, J * 72], fp16)
                tv = tt[:].rearrange("p (j r) -> p j r", j=J)
                nc.vector.tensor_add(tv, ev[:, :, 0], ev[:, :, 1])
                l1t = l1pool.tile([P, 2 * J * 72], fp16)  # (s2, j, dj, dl, h)
                a0f = l1t[:, :J * 72].rearrange("p (j r) -> p j r", j=J)
                a1f = l1t[:, J * 72:].rearrange("p (j r) -> p j r", j=J)
                nc.vector.tensor_add(a0f, tv, ev[:, :, 2])
                nc.vector.tensor_sub(a1f, ev[:, :, 2], ev[:, :, 0])

                # level 2 (contract dj) for a0 and a1 together
                lv = l1t[:].rearrange("p (s j dj r) -> p s j dj r", s=2, j=J,
                                      dj=KS)
                ut = spool.tile([P, 2 * J * 24], fp16)
                uv = ut[:].rearrange("p (s j r) -> p s j r", s=2, j=J)
                nc.vector.tensor_add(uv, lv[:, :, :, 0], lv[:, :, :, 1])
                # l3in slots: s=0: B0=sum_dj a0, s=1: C1=sum_dj a1, s=2: B1
                l3in = l3pool.tile([P, 3 * J * 24], fp16)
                sall = l3in[:, :2 * J * 24].rearrange("p (s j r) -> p s j r",
                                                      s=2, j=J)
                nc.vector.tensor_add(sall, uv, lv[:, :, :, 2])
                a0v = l1t[:, :J * 72].rearrange("p (j dj r) -> p j dj r",
                                                j=J, dj=KS)
                b1f = l3in[:, 2 * J * 24:].rearrange("p (j r) -> p j r", j=J)
                nc.vector.tensor_sub(b1f, a0v[:, :, 2], a0v[:, :, 0])

                # level 3 (contract dl): zt slots = (s0, N_di, N_dj, N_dl)
                l3v = l3in[:].rearrange("p (s j dl h) -> p s j dl h", s=3,
                                        j=J, dl=KS)
                wt = spool.tile([P, 3 * J * NH], fp16)
                wv = wt[:].rearrange("p (s j h) -> p s j h", s=3, j=J)
                nc.vector.tensor_add(wv, l3v[:, :, :, 0], l3v[:, :, :, 1])
                zt = spool.tile([P, 4 * J * NH], fp32)
                zv = zt[:, :3 * J * NH].rearrange("p (s j h) -> p s j h",
                                                  s=3, j=J)
                nc.vector.tensor_add(zv, wv, l3v[:, :, :, 2])
                b0v = l3v[:, 0]  # [p, j, dl, h]
                ndl = zt[:, 3 * J * NH:].rearrange("p (j h) -> p j h", j=J)
                nc.vector.tensor_sub(ndl, b0v[:, :, 2], b0v[:, :, 0])

                # out[o, j, h] = N_o * (1/s0); the reciprocal is cheap on
                # DVE, the final scale runs on GPSIMD off the Vector
                # critical path (out-DMA chains on the same engine)
                rt = spool.tile([P, J * NH], fp32)
                nc.vector.reciprocal_approx_fast(rt[:], zt[:, :J * NH])
                ot = opool.tile([P, 3 * TPP * NH], fp32)
                r_b = (rt[:].rearrange("p (j h) -> p j h", j=J)
                       .unsqueeze(1).broadcast_to([P, 3, J, NH]))
                nc.vector.tensor_mul(
                    ot[:].rearrange("p (o j h) -> p o j h", o=3, j=J),
                    zt[:, J * NH:].rearrange("p (o j h) -> p o j h", o=3,
                                             j=J),
                    r_b,
                )
                nc.gpsimd.dma_start(out[ti], ot[:])

            for ti in range(TILES):
                emit_front(ti)
                if ti >= 1:
                    emit_back(ti - 1)
            emit_back(TILES - 1)

    nc.compile()
    return nc


def _host_prep(q, k, rpb):
    q = np.asarray(q, dtype=np.float32)
    k = np.asarray(k, dtype=np.float32)
    rpb = np.asarray(rpb, dtype=np.float32)

    q0 = (q[0] * SCALE).astype(F16)                 # [40,40,40,48]
    kp = np.pad(k[0], ((1, 1), (1, 1), (1, 1), (0, 0)))  # [42,42,42,48]
    win = np.lib.stride_tricks.sliding_window_view(kp, (KS, KS, KS),
                                                   axis=(0, 1, 2))
    # win: [40,40,40,48,3,3,3] -> [40,40,40,kk,48]
    win = win.transpose(0, 1, 2, 4, 5, 6, 3).reshape(H, W, T, NT, DIM)

    rpb_kh = np.ascontiguousarray(rpb.reshape(NH, NT).T).reshape(FKH)
    rpb_t = np.broadcast_to(rpb_kh.astype(F16), (P, FKH)).copy()
    ident = np.eye(P, dtype=F16)

    in_maps = []
    for i in range(N_CORES):
        h0 = i * SLAB
        # tokens -> (tile, partition, j); free layouts are d-major
        q_pad = np.zeros((TOKP, NH, HD), F16)
        q_pad[:TOK] = q0[h0:h0 + SLAB].reshape(TOK, NH, HD)
        # [ti, p, j, h, d] -> [ti, p, d, j, h]
        q_t = np.ascontiguousarray(
            q_pad.reshape(TILES, P, TPP, NH, HD).transpose(0, 1, 4, 2, 3)
        ).reshape(TILES, P, HD * TPP * NH)

        kn_pad = np.zeros((TOKP, NT, NH, HD), F16)
        kn_pad[:TOK] = win[h0:h0 + SLAB].reshape(TOK, NT, NH, HD)
        # [ti, p, j, kk, h, d] -> [ti, p, d, j, kk, h]
        kn_t = np.ascontiguousarray(
            kn_pad.reshape(TILES, P, TPP, NT, NH, HD)
            .transpose(0, 1, 5, 2, 3, 4)
        ).reshape(TILES, P, HD * TPP * NT * NH)

        in_maps.append({"qs": q_t, "kn": kn_t, "rpbt": rpb_t,
                        "ident": ident})
    return in_maps


def _assemble(results):
    slabs = []
    for i in range(N_CORES):
        o = results[i]["out"].reshape(TILES, P, 3, TPP, NH)
        o = o.transpose(0, 1, 3, 2, 4).reshape(TOKP, 3, NH)[:TOK]
        o = o.reshape(SLAB, W, T, 3, NH)
        # channel order in reference: c = h*3 + o
        slabs.append(o.transpose(0, 1, 2, 4, 3).reshape(SLAB, W, T, 3 * NH))
    full = np.concatenate(slabs, axis=0)             # [40,40,40,24]
    return np.ascontiguousarray(full.transpose(3, 0, 1, 2))[None]


def _run(q, k, rpb, **spmd_kwargs):
    if "prog" not in _prog_cache:
        _prog_cache["prog"] = _build_program()
    nc = _prog_cache["prog"]
    in_maps = _host_prep(q, k, rpb)
    res = run_bass_kernel_spmd(nc, in_maps, list(range(N_CORES)),
                               **spmd_kwargs)
    return _assemble(res.results), res


def kernel(q, k, rpb):
    out, _ = _run(q, k, rpb)
    return out


# revision 21
# speedup vs baseline: 1.0441x; 1.0441x over previous
"""Trainium2 Bass kernel for 3D neighborhood attention (sparse_attention).

Problem: q,k [1,40,40,40,48] fp32, rpb [8,3,3,3]; out [1,24,40,40,40].
Per voxel x: logits[h,kk] = scale * <q[x,h,:], k[x+off_kk,h,:]> + rpb[h,kk]
(zero-padded k at boundaries, kk over 3x3x3 offsets), p = softmax over kk,
out[x,h,:] = sum_kk p[h,kk] * off_kk  (constant integer offsets as values).

Sharding: spatial-parallel over H (40 -> 8 slabs of 5). Each core gets its
q slab plus a host-side im2col of the 27 shifted k views for its slab
(halo handled on host): on-core everything is token-parallel with tokens
on SBUF partitions (9 tokens per partition, 7 tiles of 1152 tokens).

Engine split:
 - DVE: QK products in fp16 (2x_1P packed mode), then the factorized
   (di -> dj -> dl) fold tree over exp'd logits that yields the softmax
   denominator and the three directional numerators (values are the
   constant offsets in {-1,0,1}^3), plus reciprocal + final scale.
 - TensorE: the d-reduction of the products as accumulating identity
   matmuls into PSUM (6 d-blocks + an rpb seed per 432-wide chunk), so
   logits materialize in PSUM in fp32 for free.
 - ScalarE: exp straight from PSUM into fp16 SBUF.
 The DVE fold tree for tile i is emitted during tile i+1 (software
 pipelining) so the in-order DVE stream never stalls on the
 TensorE->ScalarE chain; the output DMA is dispatched from the GpSimd
 DGE queue so it never head-of-line-blocks the input DMAs on the sync
 queue; tile 0's k-neighborhood DMA is split per d-block to shorten the
 initial ramp.
"""

import numpy as np

import concourse.bass as bass
import concourse.tile as tile
from concourse import bacc, mybir
from concourse.bass_utils import run_bass_kernel_spmd

F16 = np.float16

NH = 8
HD = 6
DIM = NH * HD
KS = 3
NT = KS**3  # 27
SCALE = HD**-0.5
H = W = T = 40
N_CORES = 8
SLAB = H // N_CORES          # 5 rows of H per core
TOK = SLAB * W * T           # 8000 tokens per core
P = 128
TPP = 9                      # tokens per partition per tile
TILES = 7                    # 7 * 128 * 9 = 8064 >= 8000
TOKP = TILES * P * TPP       # 8064
FKH = NT * NH                # 216 logits per token
NCHUNK = 4                   # PSUM chunks per tile
CHUNK = TPP * FKH // NCHUNK  # 486 logits (<=512 fp32, one PSUM bank)

_prog_cache = {}


def _build_program():
    fp16 = mybir.dt.float16
    fp32 = mybir.dt.float32
    nc = bacc.Bacc("TRN2", target_bir_lowering=False, debug=False,
                   num_devices=N_CORES)
    # free layouts (per partition):
    #   qs : (d6, j8, h8)             = 384
    #   kn : (d6, j8, kk27, h8)       = 10368
    #   rpb: (kk27, h8)               = 216
    #   out: (o3, j8, h8)             = 192
    qs = nc.dram_tensor("qs", [TILES, P, HD * TPP * NH], fp16,
                        kind="ExternalInput").ap()
    kn = nc.dram_tensor("kn", [TILES, P, HD * TPP * NT * NH], fp16,
                        kind="ExternalInput").ap()
    rpbt = nc.dram_tensor("rpbt", [P, FKH], fp16, kind="ExternalInput").ap()
    ident_in = nc.dram_tensor("ident", [P, P], fp16,
                              kind="ExternalInput").ap()
    out = nc.dram_tensor("out", [TILES, P, 3 * TPP * NH], fp32,
                         kind="ExternalOutput").ap()

    J = TPP
    NJH = TPP * NT * NH         # 1728 logits per partition
    D3 = 3 * NJH                # 5184: half of the products

    with tile.TileContext(nc) as tc:
        with (
            tc.tile_pool(name="consts", bufs=1) as cpool,
            tc.tile_pool(name="kin", bufs=4) as kpool,
            tc.tile_pool(name="qin", bufs=2) as qpool,
            tc.tile_pool(name="prod", bufs=2) as ppool,
            tc.tile_pool(name="psum", bufs=8, space="PSUM") as pspool,
            tc.tile_pool(name="expv", bufs=2) as epool,
            tc.tile_pool(name="l1", bufs=2) as l1pool,
            tc.tile_pool(name="l3", bufs=2) as l3pool,
            tc.tile_pool(name="tt", bufs=2) as ttpool,
            tc.tile_pool(name="small", bufs=8) as spool,
            tc.tile_pool(name="outp", bufs=2) as opool,
        ):
            rpb_sb = cpool.tile([P, FKH], fp16)
            nc.sync.dma_start(rpb_sb[:], rpbt[:])
            ident = cpool.tile([P, P], fp16)
            nc.sync.dma_start(ident[:], ident_in[:])
            # rpb replicated across all tokens of a tile; chunk seeds are
            # 486-wide slices of this
            rpb_rep = cpool.tile([P, TPP * FKH], fp16)
            nc.vector.tensor_copy(
                rpb_rep[:].rearrange("p (j f) -> p j f", j=TPP),
                rpb_sb[:].unsqueeze(1).broadcast_to([P, TPP, FKH]))

            state = {}

            def emit_front(ti):
                """DMA + QK mul + TensorE d-fold + exp for tile ti."""
                qt = qpool.tile([P, HD * TPP * NH], fp16)
                nc.sync.dma_start(qt[:], qs[ti])
                # tile 0 loads per-d-block so the first mul starts as soon
                # as possible; later tiles use halves (fewer ops)
                nparts = HD if ti == 0 else 2
                dpp = HD // nparts            # d-blocks per part
                PSZ = dpp * NJH
                ktiles = []
                for pi in range(nparts):
                    ktp = kpool.tile([P, PSZ], fp16)
                    nc.sync.dma_start(
                        ktp[:], kn[ti, :, pi * PSZ:(pi + 1) * PSZ])
                    ktiles.append(ktp)

                pt = ppool.tile([P, HD * NJH], fp16)
                qv = qt[:].rearrange("p (d j h) -> p d j h", d=HD, j=J)
                for pi, ktile in enumerate(ktiles):
                    q_b = (qv[:, dpp * pi:dpp * (pi + 1)]
                           .unsqueeze(3).broadcast_to([P, dpp, J, NT, NH]))
                    nc.vector.tensor_mul(
                        pt[:, pi * PSZ:(pi + 1) * PSZ].rearrange(
                            "p (d j kk h) -> p d j kk h", d=dpp, j=J, kk=NT),
                        ktile[:].rearrange(
                            "p (d j kk h) -> p d j kk h", d=dpp, j=J, kk=NT),
                        q_b,
                    )
                # logits into PSUM: per chunk, seed with rpb then accumulate
                # the 6 d-blocks via identity matmuls
                et = epool.tile([P, NJH], fp16)
                for c in range(NCHUNK):
                    pc = pspool.tile([P, CHUNK], fp32)
                    nc.tensor.matmul(
                        pc[:], ident[:],
                        rpb_rep[:, c * CHUNK:(c + 1) * CHUNK],
                        start=True, stop=False)
                    for b in range(HD):
                        nc.tensor.matmul(
                            pc[:], ident[:],
                            pt[:, b * NJH + c * CHUNK:
                                b * NJH + (c + 1) * CHUNK],
                            start=False, stop=(b == HD - 1))
                    nc.scalar.activation(et[:, c * CHUNK:(c + 1) * CHUNK],
                                         pc[:],
                                         mybir.ActivationFunctionType.Exp)
                state[ti] = et

            def emit_back(ti):
                """DVE fold tree + out for tile ti (runs one tile late)."""
                et = state.pop(ti)
                # level 1 (contract di): a0 = sum_di E, a1 = E[di2]-E[di0]
                ev = et[:].rearrange("p (j di r) -> p j di r", j=J, di=KS)
                tt = ttpool.tile([P, J * 72], fp16)
                tv = tt[:].rearrange("p (j r) -> p j r", j=J)
                nc.vector.tensor_add(tv, ev[:, :, 0], ev[:, :, 1])
                l1t = l1pool.tile([P, 2 * J * 72], fp16)  # (s2, j, dj, dl, h)
                a0f = l1t[:, :J * 72].rearrange("p (j r) -> p j r", j=J)
                a1f = l1t[:, J * 72:].rearrange("p (j r) -> p j r", j=J)
                nc.vector.tensor_add(a0f, tv, ev[:, :, 2])
                nc.vector.tensor_sub(a1f, ev[:, :, 2], ev[:, :, 0])

                # level 2 (contract dj) for a0 and a1 together
                lv = l1t[:].rearrange("p (s j dj r) -> p s j dj r", s=2, j=J,
                                      dj=KS)
                ut = spool.tile([P, 2 * J * 24], fp16)
                uv = ut[:].rearrange("p (s j r) -> p s j r", s=2, j=J)
                nc.vector.tensor_add(uv, lv[:, :, :, 0], lv[:, :, :, 1])
                # l3in slots: s=0: B0=sum_dj a0, s=1: C1=sum_dj a1, s=2: B1
                l3in = l3pool.tile([P, 3 * J * 24], fp16)
                sall = l3in[:, :2 * J * 24].rearrange("p (s j r) -> p s j r",
                                                      s=2, j=J)
                nc.vector.tensor_add(sall, uv, lv[:, :, :, 2])
                a0v = l1t[:, :J * 72].rearrange("p (j dj r) -> p j dj r",
                                                j=J, dj=KS)
                b1f = l3in[:, 2 * J * 24:].rearrange("p (j r) -> p j r", j=J)
                nc.vector.tensor_sub(b1f, a0v[:, :, 2], a0v[:, :, 0])

                # level 3 (contract dl): zt slots = (s0, N_di, N_dj, N_dl)
                l3v = l3in[:].rearrange("p (s j dl h) -> p s j dl h", s=3,
                                        j=J, dl=KS)
                wt = spool.tile([P, 3 * J * NH], fp16)
                wv = wt[:].rearrange("p (s j h) -> p s j h", s=3, j=J)
                nc.vector.tensor_add(wv, l3v[:, :, :, 0], l3v[:, :, :, 1])
                zt = spool.tile([P, 4 * J * NH], fp32)
                zv = zt[:, :3 * J * NH].rearrange("p (s j h) -> p s j h",
                                                  s=3, j=J)
                nc.vector.tensor_add(zv, wv, l3v[:, :, :, 2])
                b0v = l3v[:, 0]  # [p, j, dl, h]
                ndl = zt[:, 3 * J * NH:].rearrange("p (j h) -> p j h", j=J)
                nc.vector.tensor_sub(ndl, b0v[:, :, 2], b0v[:, :, 0])

                # out[o, j, h] = N_o * (1/s0); the reciprocal is cheap on
                # DVE, the final scale runs on GPSIMD off the Vector
                # critical path (out-DMA chains on the same engine)
                rt = spool.tile([P, J * NH], fp32)
                nc.vector.reciprocal_approx_fast(rt[:], zt[:, :J * NH])
                ot = opool.tile([P, 3 * TPP * NH], fp32)
                r_b = (rt[:].rearrange("p (j h) -> p j h", j=J)
                       .unsqueeze(1).broadcast_to([P, 3, J, NH]))
                nc.vector.tensor_mul(
                    ot[:].rearrange("p (o j h) -> p o j h", o=3, j=J),
                    zt[:, J * NH:].rearrange("p (o j h) -> p o j h", o=3,
                                             j=J),
                    r_b,
                )
                nc.gpsimd.dma_start(out[ti], ot[:])

            for ti in range(TILES):
                emit_front(ti)
                if ti >= 1:
                    emit_back(ti - 1)
            emit_back(TILES - 1)

    nc.compile()
    return nc


def _host_prep(q, k, rpb):
    q = np.asarray(q, dtype=np.float32)
    k = np.asarray(k, dtype=np.float32)
    rpb = np.asarray(rpb, dtype=np.float32)

    q0 = (q[0] * SCALE).astype(F16)                 # [40,40,40,48]
    kp = np.pad(k[0], ((1, 1), (1, 1), (1, 1), (0, 0)))  # [42,42,42,48]
    win = np.lib.stride_tricks.sliding_window_view(kp, (KS, KS, KS),
                                                   axis=(0, 1, 2))
    # win: [40,40,40,48,3,3,3] -> [40,40,40,kk,48]
    win = win.transpose(0, 1, 2, 4, 5, 6, 3).reshape(H, W, T, NT, DIM)

    rpb_kh = np.ascontiguousarray(rpb.reshape(NH, NT).T).reshape(FKH)
    rpb_t = np.broadcast_to(rpb_kh.astype(F16), (P, FKH)).copy()
    ident = np.eye(P, dtype=F16)

    in_maps = []
    for i in range(N_CORES):
        h0 = i * SLAB
        # tokens -> (tile, partition, j); free layouts are d-major
        q_pad = np.zeros((TOKP, NH, HD), F16)
        q_pad[:TOK] = q0[h0:h0 + SLAB].reshape(TOK, NH, HD)
        # [ti, p, j, h, d] -> [ti, p, d, j, h]
        q_t = np.ascontiguousarray(
            q_pad.reshape(TILES, P, TPP, NH, HD).transpose(0, 1, 4, 2, 3)
        ).reshape(TILES, P, HD * TPP * NH)

        kn_pad = np.zeros((TOKP, NT, NH, HD), F16)
        kn_pad[:TOK] = win[h0:h0 + SLAB].reshape(TOK, NT, NH, HD)
        # [ti, p, j, kk, h, d] -> [ti, p, d, j, kk, h]
        kn_t = np.ascontiguousarray(
            kn_pad.reshape(TILES, P, TPP, NT, NH, HD)
            .transpose(0, 1, 5, 2, 3, 4)
        ).reshape(TILES, P, HD * TPP * NT * NH)

        in_maps.append({"qs": q_t, "kn": kn_t, "rpbt": rpb_t,
                        "ident": ident})
    return in_maps


def _assemble(results):
    slabs = []
    for i in range(N_CORES):
        o = results[i]["out"].reshape(TILES, P, 3, TPP, NH)
        o = o.transpose(0, 1, 3, 2, 4).reshape(TOKP, 3, NH)[:TOK]
        o = o.reshape(SLAB, W, T, 3, NH)
        # channel order in reference: c = h*3 + o
        slabs.append(o.transpose(0, 1, 2, 4, 3).reshape(SLAB, W, T, 3 * NH))
    full = np.concatenate(slabs, axis=0)             # [40,40,40,24]
    return np.ascontiguousarray(full.transpose(3, 0, 1, 2))[None]


def _run(q, k, rpb, **spmd_kwargs):
    if "prog" not in _prog_cache:
        _prog_cache["prog"] = _build_program()
    nc = _prog_cache["prog"]
    in_maps = _host_prep(q, k, rpb)
    res = run_bass_kernel_spmd(nc, in_maps, list(range(N_CORES)),
                               **spmd_kwargs)
    return _assemble(res.results), res


def kernel(q, k, rpb):
    out, _ = _run(q, k, rpb)
    return out
